# revision 1
# baseline (speedup 1.0000x reference)
"""Trainium2 Bass kernel for the differentiable Gaussian renderer.

Math: for each pose, each gaussian g splats w[g,p] = op_g * exp(-0.5*d2/var_g)
onto pixels p; output = (sum_g w*color) / (sum_g w + n_chunks*eps), tiled.

Key structure exploited: the Gaussian is separable, exp(-(dx^2+dy^2)*s) =
Ex(c) * Ey(r), where dx depends only on the pixel column and dy only on the
row.  Per gaussian we need just 256 exp evaluations instead of 16384, and the
pixel accumulation becomes, per 128-gaussian chunk, one K=128 matmul:

  acc[r, ch*128+c] += sum_g Ey[g,r] * rhs[g, ch*128+c],
  rhs[g, ch*128+c] = Ex[g,c] * colors[g, ch]   (ch==3 column is Ex itself,
                                                giving the denominator)

The exp arguments arg_x[g,c] = niv_g*(c'-u'_g)^2 + ln(op_g) (and arg_y) are
produced on the tensor engine: per 4-chunk block, the per-gaussian bf16
coefficients (each split 3-way hi/mid/lo for ~1e-4 absolute accuracy) are
PE-transposed into a [32*4, 128] layout, then TWO full-array K=128 bf16
matmuls against block-diagonal constant matrices of {1, c', c'^2} rows
produce all 4 chunks' x-args and y-args (one PSUM bank each).  Opacity rides
in ln-space inside arg_x; u,v are clamped to +-110.5 around the image center
(gaussians beyond that have w == 0 in fp32 anyway).

Sharding: gaussians are split 8 ways (8192/core).  Each core renders partial
num/den for both poses; per (pose, half-of-gaussians) AllReduce(add) combines
them across cores (3 of the 4 reductions overlap rendering), every core then
divides and writes the full [2,3,128,128] image planes; the host reshapes to
the reference tiling.

Pipeline: per 4-chunk block, phase A = PE transpose -> 2 arg-MMs -> ACT exp
-> color scale (broadcast tensor_tensor alternating DVE / GPSIMD); the
block's 4 f32r main-MMs are emitted one block behind, so the PE always has
phase-A work while colors finish, keeping the HAM clock gate warm.
"""
import numpy as np
import ml_dtypes

import concourse.mybir as mybir
import concourse.tile as tile
import concourse.bacc as bacc
from concourse.bass_utils import run_bass_kernel_spmd

f32 = mybir.dt.float32
f32r = mybir.dt.float32r
bf16 = mybir.dt.bfloat16
ALU = mybir.AluOpType
ACTF = mybir.ActivationFunctionType

NCORES = 8
NPOSE = 2
H = W = 128
FX = FY = 120.0
CX = CY = 64.0
NG = 65536
NGC = NG // NCORES          # gaussians per core
NCHUNK = NGC // 128         # 64 chunks of 128 gaussians
NBLK = NCHUNK // 4          # 16 transpose blocks of 4 chunks
CENT = 63.5
UCLAMP = 110.5

# q-slot layout inside each chunk's 32 coefficient rows (see _const_blocks)
NIV_X = (12, 13, 14, 15, 16, 17)
NIV_Y = (18, 19, 20, 21, 22, 23)


def _bf(x):
    return np.asarray(x).astype(ml_dtypes.bfloat16)


def _split3(x):
    h = _bf(x).astype(np.float64)
    m = _bf(x - h).astype(np.float64)
    l = _bf(x - h - m).astype(np.float64)
    return h, m, l


def _const_blocks():
    """(constX, constY): [128, 512] bf16 block-diagonal matmul constants.
    Block jj (rows 32jj.., cols 128jj..) holds the per-q constant rows for
    chunk jj of a 4-chunk transpose block."""
    cp = np.arange(128, dtype=np.float64) - CENT
    c2 = cp * cp
    c2h, c2m, c2l = _split3(c2)
    ones = np.ones(128)
    zer = np.zeros(128)
    xrows, yrows = [], []
    for _ in range(3):                       # h / m / l coefficient groups
        xrows += [ones, cp, zer, zer]
        yrows += [zer, zer, ones, cp]
    xrows += [c2h, c2m, c2l, c2h, c2m, c2h] + [zer] * 6 + [zer] * 8
    yrows += [zer] * 6 + [c2h, c2m, c2l, c2h, c2m, c2h] + [zer] * 8
    bx, by = np.stack(xrows), np.stack(yrows)
    cx = np.zeros((128, 512))
    cy = np.zeros((128, 512))
    for jj in range(4):
        cx[32 * jj:32 * jj + 32, 128 * jj:128 * jj + 128] = bx
        cy[32 * jj:32 * jj + 32, 128 * jj:128 * jj + 128] = by
    return _bf(cx), _bf(cy)


def _quat2mat(q):
    q = np.asarray(q, np.float64)
    q = q / np.linalg.norm(q)
    w, x, y, z = q
    return np.array([
        [1 - 2 * (y * y + z * z), 2 * (x * y - z * w), 2 * (x * z + y * w)],
        [2 * (x * y + z * w), 1 - 2 * (x * x + z * z), 2 * (y * z - x * w)],
        [2 * (x * z - y * w), 2 * (y * z + x * w), 1 - 2 * (x * x + y * y)],
    ])


def _build(eps_total: float, use_collective: bool = True):
    nc = bacc.Bacc("TRN2", target_bir_lowering=False, debug=False,
                   num_devices=NCORES)
    # host pre-laid-out inputs: partition p holds gaussian j*128+p at free j
    pos = nc.dram_tensor("positions", [128, NCHUNK, 3], f32, kind="ExternalInput")
    col = nc.dram_tensor("colors", [128, NCHUNK, 3], f32, kind="ExternalInput")
    opa = nc.dram_tensor("opacities", [128, NCHUNK], f32, kind="ExternalInput")
    sca = nc.dram_tensor("scales", [128, NCHUNK], f32, kind="ExternalInput")
    rt = nc.dram_tensor("rt", [NPOSE, 12], f32, kind="ExternalInput")
    out = nc.dram_tensor("out", [NPOSE, 3, H, W], f32, kind="ExternalOutput")

    cxb, cyb = _const_blocks()
    constx_d = nc.inline_tensor(np.asarray(cxb), name="constX")
    consty_d = nc.inline_tensor(np.asarray(cyb), name="constY")
    ident_d = nc.inline_tensor(np.eye(128, dtype=ml_dtypes.bfloat16), name="ident")

    with tile.TileContext(nc) as tc:
        with (
            tc.tile_pool(name="const", bufs=1) as cpool,
            tc.tile_pool(name="prep", bufs=2) as prep,
            tc.tile_pool(name="pk", bufs=2) as pkpool,
            tc.tile_pool(name="t32", bufs=3) as t32pool,
            tc.tile_pool(name="rhs4", bufs=3) as rhspool,
            tc.tile_pool(name="ey4", bufs=3) as eypool,
            tc.tile_pool(name="fin", bufs=1) as fin,
            tc.tile_pool(name="ps_tr", bufs=2, space="PSUM") as ps_tr,
            tc.tile_pool(name="ps_arg", bufs=2, space="PSUM") as ps_arg,
            tc.tile_pool(name="ps_acc", bufs=2, space="PSUM") as ps_acc,
            tc.tile_pool(name="dram", bufs=1, space="DRAM") as dpool,
        ):
            # ---- constants / inputs to SBUF (contiguous DMAs) ----
            constx = cpool.tile([128, 512], bf16)
            nc.sync.dma_start(constx[:], constx_d.ap())
            consty = cpool.tile([128, 512], bf16)
            nc.sync.dma_start(consty[:], consty_d.ap())
            ident = cpool.tile([128, 128], bf16)
            nc.sync.dma_start(ident[:], ident_d.ap())
            pos_t = cpool.tile([128, NCHUNK, 3], f32)
            nc.sync.dma_start(pos_t[:], pos.ap())
            col3 = cpool.tile([128, NCHUNK, 3], f32)
            nc.sync.dma_start(col3[:], col.ap())
            opat = cpool.tile([128, NCHUNK], f32)
            nc.sync.dma_start(opat[:], opa.ap())
            scat = cpool.tile([128, NCHUNK], f32)
            nc.sync.dma_start(scat[:], sca.ap())
            rtb = cpool.tile([128, NPOSE * 12], f32)
            nc.sync.dma_start(rtb[:], rt.ap().rearrange("a b -> (a b)")
                              .partition_broadcast(128))

            # ---- pose-independent per-gaussian prep ----
            opc = prep.tile([128, NCHUNK], f32, tag="opc")
            nc.vector.tensor_scalar_max(opc[:], opat[:], 1e-30)
            lnop = prep.tile([128, NCHUNK], f32, tag="lnop")
            nc.scalar.activation(lnop[:], opc[:], ACTF.Ln)

            s2 = prep.tile([128, NCHUNK], f32, tag="s2")
            nc.vector.tensor_tensor(s2[:], scat[:], scat[:], ALU.mult)
            m2s2 = prep.tile([128, NCHUNK], f32, tag="m2s2")
            nc.vector.tensor_scalar_mul(m2s2[:], s2[:], -2.0)
            niv = prep.tile([128, NCHUNK], f32, tag="niv")
            nc.vector.reciprocal(niv[:], m2s2[:])

            nivh = prep.tile([128, NCHUNK], bf16, tag="nivh")
            nc.vector.tensor_copy(nivh[:], niv[:])
            r1 = prep.tile([128, NCHUNK], f32, tag="r1")
            nc.vector.tensor_tensor(r1[:], niv[:], nivh[:], ALU.subtract)
            nivm = prep.tile([128, NCHUNK], bf16, tag="nivm")
            nc.vector.tensor_copy(nivm[:], r1[:])
            r2 = prep.tile([128, NCHUNK], f32, tag="r2")
            nc.vector.tensor_tensor(r2[:], r1[:], nivm[:], ALU.subtract)
            nivl = prep.tile([128, NCHUNK], bf16, tag="nivl")
            nc.vector.tensor_copy(nivl[:], r2[:])

            bnc_in = dpool.tile([NPOSE, 2, 128, 512], f32)
            bnc_out = dpool.tile([NPOSE, 2, 128, 512], f32)

            def rsc(p, k):
                return rtb[:, p * 12 + k: p * 12 + k + 1]

            def build_packed(p):
                """Per-pose packed coefficient tile (bf16, [128,NCHUNK,32])."""
                packed = pkpool.tile([128, NCHUNK, 32], bf16, tag="pk",
                                     name=f"packed{p}")
                nc.gpsimd.memset(packed[:, :, 24:32], 0.0)
                for q in (NIV_X[0], NIV_X[1], NIV_X[2],
                          NIV_Y[0], NIV_Y[1], NIV_Y[2]):
                    nc.gpsimd.tensor_copy(packed[:, :, q], nivh[:])
                for q in (NIV_X[3], NIV_X[4], NIV_Y[3], NIV_Y[4]):
                    nc.gpsimd.tensor_copy(packed[:, :, q], nivm[:])
                for q in (NIV_X[5], NIV_Y[5]):
                    nc.gpsimd.tensor_copy(packed[:, :, q], nivl[:])

                cam = []
                for crow in range(3):
                    acc = prep.tile([128, NCHUNK], f32, tag=f"cam{crow}")
                    nc.vector.tensor_scalar(acc[:], pos_t[:, :, 0],
                                            rsc(p, 3 * crow + 0),
                                            rsc(p, 9 + crow), ALU.mult, ALU.add)
                    for k in (1, 2):
                        t_ = prep.tile([128, NCHUNK], f32, tag="cam_t")
                        nc.vector.tensor_scalar_mul(t_[:], pos_t[:, :, k],
                                                    rsc(p, 3 * crow + k))
                        nc.vector.tensor_tensor(acc[:], acc[:], t_[:], ALU.add)
                    cam.append(acc)

                zr = prep.tile([128, NCHUNK], f32, tag="zr")
                nc.vector.reciprocal(zr[:], cam[2][:])

                cf = prep.tile([128, NCHUNK, 4], f32, tag="cf")
                for ax_i, (ci, foc, off) in enumerate(
                        ((cam[0], FX, CX), (cam[1], FY, CY))):
                    t_ = prep.tile([128, NCHUNK], f32, tag="uv_t")
                    nc.vector.tensor_tensor(t_[:], ci[:], zr[:], ALU.mult)
                    u_ = prep.tile([128, NCHUNK], f32, tag="uv_u")
                    nc.vector.tensor_scalar(u_[:], t_[:], float(foc),
                                            float(off - CENT), ALU.mult, ALU.add)
                    ucl = prep.tile([128, NCHUNK], f32, tag="uv_ucl")
                    nc.vector.tensor_scalar(ucl[:], u_[:], -UCLAMP, UCLAMP,
                                            ALU.max, ALU.min)
                    w2 = prep.tile([128, NCHUNK], f32, tag="w2")
                    nc.vector.tensor_tensor(w2[:], ucl[:], ucl[:], ALU.mult)
                    a_v = cf[:, :, 0 + 2 * ax_i]
                    nc.vector.tensor_tensor(a_v, w2[:], niv[:], ALU.mult)
                    if ax_i == 0:
                        nc.vector.tensor_tensor(a_v, a_v, lnop[:], ALU.add)
                    b_f = prep.tile([128, NCHUNK], f32, tag="b_f")
                    nc.vector.tensor_tensor(b_f[:], ucl[:], niv[:], ALU.mult)
                    nc.vector.tensor_scalar_mul(cf[:, :, 1 + 2 * ax_i],
                                                b_f[:], -2.0)

                nc.vector.tensor_copy(packed[:, :, 0:4], cf[:])
                sr1 = prep.tile([128, NCHUNK, 4], f32, tag="sr1")
                nc.vector.tensor_tensor(sr1[:], cf[:], packed[:, :, 0:4],
                                        ALU.subtract)
                nc.vector.tensor_copy(packed[:, :, 4:8], sr1[:])
                sr2 = prep.tile([128, NCHUNK, 4], f32, tag="sr2")
                nc.vector.tensor_tensor(sr2[:], sr1[:], packed[:, :, 4:8],
                                        ALU.subtract)
                nc.vector.tensor_copy(packed[:, :, 8:12], sr2[:])
                return packed

            # prep for BOTH poses up front so pose 1's coefficients are ready
            # before the PE reaches them (no mid-kernel stall)
            packed_all = [build_packed(p) for p in range(NPOSE)]

            def phase_a(p, packed, bb):
                """Transpose + arg MMs + exp + colors for 4-chunk block bb.
                Returns (ey4, rhs4) tiles."""
                ptr = ps_tr.tile([128, 128], bf16, tag="tr")
                nc.tensor.transpose(
                    ptr[:], packed[:, 4 * bb: 4 * bb + 4, :]
                    .rearrange("p a b -> p (a b)"), ident[:])
                t32 = t32pool.tile([128, 128], bf16, tag="t32")
                nc.scalar.copy(t32[:], ptr[:])

                parg = ps_arg.tile([128, 1024], f32, tag="arg")
                nc.tensor.matmul(parg[:, 0:512], t32[:], constx[:],
                                 start=True, stop=True)
                nc.tensor.matmul(parg[:, 512:1024], t32[:], consty[:],
                                 start=True, stop=True)
                rhs4 = rhspool.tile([128, 4, 512], f32r, tag="rhs4")
                ey4 = eypool.tile([128, 4, 128], f32r, tag="ey4")
                nc.scalar.activation(rhs4[:, :, 384:512],
                                     parg[:, 0:512]
                                     .rearrange("p (a x) -> p a x", a=4),
                                     ACTF.Exp)
                nc.scalar.activation(ey4[:, :, :],
                                     parg[:, 512:1024]
                                     .rearrange("p (a x) -> p a x", a=4),
                                     ACTF.Exp)
                nc.vector.tensor_tensor(
                    rhs4[:, :, 0:384].rearrange("p a (c x) -> p a c x", c=3),
                    rhs4[:, :, 384:512].unsqueeze(2)
                    .broadcast_to([128, 4, 3, 128]),
                    col3[:, 4 * bb: 4 * bb + 4, :].unsqueeze(3)
                    .broadcast_to([128, 4, 3, 128]),
                    ALU.mult)
                return ey4, rhs4

            finals = []
            for p in range(NPOSE):
                packed = packed_all[p]
                halves_sb = []
                for h in range(2):           # halves of the gaussian shard
                    pacc = ps_acc.tile([128, 512], f32, tag="acc")
                    pending = None
                    for b in range(NBLK // 2):
                        bb = h * (NBLK // 2) + b
                        tiles = phase_a(p, packed, bb)
                        if pending is not None:
                            ey4, rhs4 = pending
                            for k in range(4):
                                nc.tensor.matmul(
                                    pacc[:], ey4[:, k, :], rhs4[:, k, :],
                                    start=(b == 1 and k == 0), stop=False)
                        pending = tiles
                    ey4, rhs4 = pending
                    for k in range(4):
                        nc.tensor.matmul(pacc[:], ey4[:, k, :], rhs4[:, k, :],
                                         start=False, stop=(k == 3))

                    acc_sb = fin.tile([128, 512], f32, tag=f"accsb{p}{h}")
                    nc.scalar.copy(acc_sb[:], pacc[:])
                    nc.sync.dma_start(bnc_in[p, h, :, :], acc_sb[:])
                    if use_collective:
                        nc.gpsimd.collective_compute(
                            "AllReduce", ALU.add,
                            replica_groups=[list(range(NCORES))],
                            ins=[bnc_in[p, h, :, :].opt()],
                            outs=[bnc_out[p, h, :, :].opt()])
                    else:
                        nc.sync.dma_start(bnc_out[p, h, :, :],
                                          bnc_in[p, h, :, :])
                    sum_sb = fin.tile([128, 512], f32, tag=f"sum{p}{h}")
                    nc.sync.dma_start(sum_sb[:], bnc_out[p, h, :, :])
                    halves_sb.append(sum_sb)
                finals.append(halves_sb)

            # ---- final: halves add, divide, output ----
            for p in range(NPOSE):
                h0, h1 = finals[p]
                tot = fin.tile([128, 512], f32, tag=f"tot{p}")
                nc.vector.tensor_tensor(tot[:], h0[:], h1[:], ALU.add)
                dplus = fin.tile([128, 128], f32, tag=f"dplus{p}")
                nc.vector.tensor_scalar_add(dplus[:], tot[:, 384:512],
                                            float(eps_total))
                rcp = fin.tile([128, 128], f32, tag=f"rcp{p}")
                nc.vector.reciprocal(rcp[:], dplus[:])
                img = fin.tile([128, 3, 128], f32, tag=f"img{p}")
                nc.vector.tensor_tensor(
                    img[:], tot[:, 0:384].rearrange("p (c x) -> p c x", c=3),
                    rcp[:].unsqueeze(1).broadcast_to([128, 3, 128]), ALU.mult)
                nc.sync.dma_start(out.ap()[p].transpose([1, 0, 2]), img[:])

    nc.compile()
    return nc


_CACHE = {}


def _get_nc(eps_total: float):
    key = float(eps_total)
    if key not in _CACHE:
        _CACHE[key] = _build(key)
    return _CACHE[key]


def kernel(positions, colors, opacities, scales, qvec, tvec,
           tile_hw=32, chunk_gauss=4096):
    positions = np.asarray(positions, np.float32)
    colors = np.asarray(colors, np.float32)
    opacities = np.asarray(opacities, np.float32)
    scales = np.asarray(scales, np.float32)
    qvec = np.asarray(qvec, np.float32)
    tvec = np.asarray(tvec, np.float32)
    tile_hw = int(tile_hw)
    chunk_gauss = int(chunk_gauss)
    n = positions.shape[0]
    assert n == NG and tile_hw == 32, (n, tile_hw)
    eps_total = (n // chunk_gauss) * 1e-8

    rtv = np.zeros((NPOSE, 12), np.float32)
    for p in range(NPOSE):
        rtv[p, :9] = _quat2mat(qvec[p]).astype(np.float32).reshape(9)
        rtv[p, 9:12] = tvec[p]

    def lay(a, shape):
        return np.ascontiguousarray(
            a.reshape(NCHUNK, 128, -1).transpose(1, 0, 2).reshape(shape))

    in_maps = []
    for c in range(NCORES):
        sl = slice(c * NGC, (c + 1) * NGC)
        in_maps.append({
            "positions": lay(positions[sl], (128, NCHUNK, 3)),
            "colors": lay(colors[sl], (128, NCHUNK, 3)),
            "opacities": lay(opacities[sl], (128, NCHUNK)),
            "scales": lay(scales[sl], (128, NCHUNK)),
            "rt": rtv,
        })

    nc = _get_nc(eps_total)
    res = None
    for attempt in range(3):
        try:
            res = run_bass_kernel_spmd(nc, in_maps, core_ids=list(range(NCORES)))
            break
        except Exception:
            if attempt == 2:
                raise
    if res.exec_time_ns is not None:
        print(f"HW exec time: {res.exec_time_ns} ns")
    dev = res.results[0]["out"]          # [2, 3, 128, 128] (pose, ch, r, c)
    return (dev.reshape(NPOSE, 3, 16, 1024).transpose(0, 2, 1, 3)
            .reshape(NPOSE * 16, 3, tile_hw, tile_hw).astype(np.float32))



# revision 24
# speedup vs baseline: 2.2009x; 2.2009x over previous
"""Trainium2 Bass kernel for the differentiable Gaussian renderer.

Math: for each pose, each gaussian g splats w[g,p] = op_g * exp(-0.5*d2/var_g)
onto pixels p; output = (sum_g w*color) / (sum_g w + n_chunks*eps), tiled.

Key structure exploited: the Gaussian is separable, exp(-(dx^2+dy^2)*s) =
Ex(c) * Ey(r), where dx depends only on the pixel column and dy only on the
row.  Per gaussian we need just 256 exp evaluations instead of 16384, and the
pixel accumulation becomes, per 128-gaussian chunk, one K=128 matmul:

  acc[r, ch*128+c] += sum_g Ey[g,r] * rhs[g, ch*128+c],
  rhs[g, ch*128+c] = Ex[g,c] * colors[g, ch]   (ch==3 column is Ex itself,
                                                giving the denominator)

The exp arguments arg_x[g,c] = niv_g*(c'-u'_g)^2 + ln(op_g) (and arg_y) are
produced on the tensor engine: per 4-chunk block, the per-gaussian bf16
coefficients (each split 3-way hi/mid/lo for ~1e-4 absolute accuracy) are
PE-transposed into a [32*4, 128] layout, then TWO full-array K=128 bf16
matmuls against block-diagonal constant matrices of {1, c', c'^2} rows
produce all 4 chunks' x-args and y-args (one PSUM bank each).  Opacity rides
in ln-space inside arg_x; u,v are clamped to +-110.5 around the image center
(gaussians beyond that have w == 0 in fp32 anyway).

Per block, ONE fused ACT call computes exp of all x and y args (amortizes the
~400ns activation fixed cost), writing Ex (den) and Ey into one SBUF tile;
the color scaling Ex*col is split 2:1 between DVE and GPSIMD (both are
broadcast tensor_tensor, rate-limited identically by the cost model).

Sharding: gaussians are split 8 ways (8192/core).  Each core renders partial
num/den [128 rows, 512] per pose into PSUM, DMAs it straight to DRAM, and a
single ReduceScatter(add) over the concatenated [2,128,512] gives core c the
summed rows 32*(c%4).. of pose c//4.  Each core divides only its own shard
and writes a [3,32,128] output; the host assembles the 8 shards into the
full [2,3,128,128] image (run_bass_kernel_spmd returns every core's output,
so no AllGather is needed).  One collective instead of four AllReduces
removes ~145us of serialized collective time.
"""
import numpy as np
import ml_dtypes

import concourse.mybir as mybir
import concourse.tile as tile
import concourse.bacc as bacc
from concourse.bass_utils import run_bass_kernel_spmd

f32 = mybir.dt.float32
f32r = mybir.dt.float32r
bf16 = mybir.dt.bfloat16
ALU = mybir.AluOpType
ACTF = mybir.ActivationFunctionType

NCORES = 8
NPOSE = 2
H = W = 128
FX = FY = 120.0
CX = CY = 64.0
NG = 65536
NGC = NG // NCORES          # gaussians per core
NCHUNK = NGC // 128         # 64 chunks of 128 gaussians
NBLK = NCHUNK // 4          # 16 transpose blocks of 4 chunks
CENT = 63.5
UCLAMP = 110.5
RSROWS = H // NCORES              # 16 image rows per core per pose after RS

# q-slot layout inside each chunk's 32 coefficient rows (see _const_blocks)
NIV_X = (12, 13, 14, 15, 16, 17)
NIV_Y = (18, 19, 20, 21, 22, 23)


def _bf(x):
    return np.asarray(x).astype(ml_dtypes.bfloat16)


def _split3(x):
    h = _bf(x).astype(np.float64)
    m = _bf(x - h).astype(np.float64)
    l = _bf(x - h - m).astype(np.float64)
    return h, m, l


def _const_blocks():
    """(constX, constY): [128, 512] bf16 block-diagonal matmul constants.
    Block jj (rows 32jj.., cols 128jj..) holds the per-q constant rows for
    chunk jj of a 4-chunk transpose block."""
    cp = np.arange(128, dtype=np.float64) - CENT
    c2 = cp * cp
    c2h, c2m, c2l = _split3(c2)
    ones = np.ones(128)
    zer = np.zeros(128)
    xrows, yrows = [], []
    for _ in range(3):                       # h / m / l coefficient groups
        xrows += [ones, cp, zer, zer]
        yrows += [zer, zer, ones, cp]
    xrows += [c2h, c2m, c2l, c2h, c2m, c2h] + [zer] * 6 + [zer] * 8
    yrows += [zer] * 6 + [c2h, c2m, c2l, c2h, c2m, c2h] + [zer] * 8
    bx, by = np.stack(xrows), np.stack(yrows)
    cx = np.zeros((128, 512))
    cy = np.zeros((128, 512))
    for jj in range(4):
        cx[32 * jj:32 * jj + 32, 128 * jj:128 * jj + 128] = bx
        cy[32 * jj:32 * jj + 32, 128 * jj:128 * jj + 128] = by
    return _bf(cx), _bf(cy)


def _quat2mat(q):
    q = np.asarray(q, np.float64)
    q = q / np.linalg.norm(q)
    w, x, y, z = q
    return np.array([
        [1 - 2 * (y * y + z * z), 2 * (x * y - z * w), 2 * (x * z + y * w)],
        [2 * (x * y + z * w), 1 - 2 * (x * x + z * z), 2 * (y * z - x * w)],
        [2 * (x * z - y * w), 2 * (y * z + x * w), 1 - 2 * (x * x + y * y)],
    ])


def _build(eps_total: float, use_collective: bool = True):
    nc = bacc.Bacc("TRN2", target_bir_lowering=False, debug=False,
                   num_devices=NCORES)
    # host pre-laid-out inputs: partition p holds gaussian j*128+p at free j
    pos = nc.dram_tensor("positions", [128, NCHUNK, 3], f32, kind="ExternalInput")
    col = nc.dram_tensor("colors", [128, NCHUNK, 3], f32, kind="ExternalInput")
    opa = nc.dram_tensor("opacities", [128, NCHUNK], f32, kind="ExternalInput")
    sca = nc.dram_tensor("scales", [128, NCHUNK], f32, kind="ExternalInput")
    rt = nc.dram_tensor("rt", [NPOSE, 12], f32, kind="ExternalInput")
    out = nc.dram_tensor("out", [NPOSE, 3, RSROWS, W], f32,
                         kind="ExternalOutput")

    cxb, cyb = _const_blocks()
    constx_d = nc.inline_tensor(np.asarray(cxb), name="constX")
    consty_d = nc.inline_tensor(np.asarray(cyb), name="constY")
    ident_d = nc.inline_tensor(np.eye(128, dtype=ml_dtypes.bfloat16), name="ident")

    with tile.TileContext(nc) as tc:
        with (
            tc.tile_pool(name="const", bufs=1) as cpool,
            tc.tile_pool(name="prep", bufs=2) as prep,
            tc.tile_pool(name="pk", bufs=2) as pkpool,
            tc.tile_pool(name="blk", bufs=6) as blkpool,
            tc.tile_pool(name="fin", bufs=1) as fin,
            tc.tile_pool(name="ps_tr", bufs=1, space="PSUM") as ps_tr,
            tc.tile_pool(name="ps_arg", bufs=2, space="PSUM") as ps_arg,
            tc.tile_pool(name="ps_acc", bufs=2, space="PSUM") as ps_acc,
            tc.tile_pool(name="dram", bufs=1, space="DRAM") as dpool,
        ):
            # ---- constants / inputs to SBUF ----
            # spread across engine DMA queues so the HWDGE slots overlap
            constx = cpool.tile([128, 512], bf16)
            nc.sync.dma_start(constx[:], constx_d.ap())
            consty = cpool.tile([128, 512], bf16)
            nc.scalar.dma_start(consty[:], consty_d.ap())
            ident = cpool.tile([128, 128], bf16)
            nc.gpsimd.dma_start(ident[:], ident_d.ap())
            pos_t = cpool.tile([128, NCHUNK, 3], f32)
            nc.gpsimd.dma_start(pos_t[:], pos.ap())
            col3 = cpool.tile([128, NCHUNK, 3], f32)
            nc.scalar.dma_start(col3[:], col.ap())
            opat = cpool.tile([128, NCHUNK], f32)
            nc.sync.dma_start(opat[:], opa.ap())
            scat = cpool.tile([128, NCHUNK], f32)
            nc.sync.dma_start(scat[:], sca.ap())
            rtb = cpool.tile([128, NPOSE * 12], f32)
            nc.scalar.dma_start(rtb[:], rt.ap().rearrange("a b -> (a b)")
                                .partition_broadcast(128))

            # ---- pose-independent per-gaussian prep ----
            opc = prep.tile([128, NCHUNK], f32, tag="opc")
            nc.vector.tensor_scalar_max(opc[:], opat[:], 1e-30)
            lnop = prep.tile([128, NCHUNK], f32, tag="lnop")
            nc.scalar.activation(lnop[:], opc[:], ACTF.Ln)

            s2 = prep.tile([128, NCHUNK], f32, tag="s2")
            nc.vector.tensor_tensor(s2[:], scat[:], scat[:], ALU.mult)
            m2s2 = prep.tile([128, NCHUNK], f32, tag="m2s2")
            nc.vector.tensor_scalar_mul(m2s2[:], s2[:], -2.0)
            niv = prep.tile([128, NCHUNK], f32, tag="niv")
            nc.vector.reciprocal(niv[:], m2s2[:])

            nivh = prep.tile([128, NCHUNK], bf16, tag="nivh")
            nc.vector.tensor_copy(nivh[:], niv[:])
            r1 = prep.tile([128, NCHUNK], f32, tag="r1")
            nc.vector.tensor_tensor(r1[:], niv[:], nivh[:], ALU.subtract)
            nivm = prep.tile([128, NCHUNK], bf16, tag="nivm")
            nc.vector.tensor_copy(nivm[:], r1[:])
            r2 = prep.tile([128, NCHUNK], f32, tag="r2")
            nc.vector.tensor_tensor(r2[:], r1[:], nivm[:], ALU.subtract)
            nivl = prep.tile([128, NCHUNK], bf16, tag="nivl")
            nc.vector.tensor_copy(nivl[:], r2[:])

            bnc_in = dpool.tile([NPOSE, 128, 512], f32)
            bnc_out = dpool.tile([NPOSE, RSROWS, 512], f32)

            def rsc(p, k):
                return rtb[:, p * 12 + k: p * 12 + k + 1]

            def build_packed(p):
                """Per-pose packed coefficient tile (bf16, [128,NCHUNK,32])."""
                packed = pkpool.tile([128, NCHUNK, 32], bf16, tag="pk",
                                     name=f"packed{p}")
                nc.gpsimd.memset(packed[:, :, 24:32], 0.0)
                for q in (NIV_X[0], NIV_X[1], NIV_X[2],
                          NIV_Y[0], NIV_Y[1], NIV_Y[2]):
                    nc.gpsimd.tensor_copy(packed[:, :, q], nivh[:])
                for q in (NIV_X[3], NIV_X[4], NIV_Y[3], NIV_Y[4]):
                    nc.gpsimd.tensor_copy(packed[:, :, q], nivm[:])
                for q in (NIV_X[5], NIV_Y[5]):
                    nc.gpsimd.tensor_copy(packed[:, :, q], nivl[:])

                cam = []
                for crow in range(3):
                    acc = prep.tile([128, NCHUNK], f32, tag=f"cam{crow}")
                    nc.vector.tensor_scalar(acc[:], pos_t[:, :, 0],
                                            rsc(p, 3 * crow + 0),
                                            rsc(p, 9 + crow), ALU.mult, ALU.add)
                    for k in (1, 2):
                        t_ = prep.tile([128, NCHUNK], f32, tag="cam_t")
                        nc.vector.tensor_scalar_mul(t_[:], pos_t[:, :, k],
                                                    rsc(p, 3 * crow + k))
                        nc.vector.tensor_tensor(acc[:], acc[:], t_[:], ALU.add)
                    cam.append(acc)

                zr = prep.tile([128, NCHUNK], f32, tag="zr")
                nc.vector.reciprocal(zr[:], cam[2][:])

                cf = prep.tile([128, NCHUNK, 4], f32, tag="cf")
                for ax_i, (ci, foc, off) in enumerate(
                        ((cam[0], FX, CX), (cam[1], FY, CY))):
                    t_ = prep.tile([128, NCHUNK], f32, tag="uv_t")
                    nc.vector.tensor_tensor(t_[:], ci[:], zr[:], ALU.mult)
                    u_ = prep.tile([128, NCHUNK], f32, tag="uv_u")
                    nc.vector.tensor_scalar(u_[:], t_[:], float(foc),
                                            float(off - CENT), ALU.mult, ALU.add)
                    ucl = prep.tile([128, NCHUNK], f32, tag="uv_ucl")
                    nc.vector.tensor_scalar(ucl[:], u_[:], -UCLAMP, UCLAMP,
                                            ALU.max, ALU.min)
                    w2 = prep.tile([128, NCHUNK], f32, tag="w2")
                    nc.vector.tensor_tensor(w2[:], ucl[:], ucl[:], ALU.mult)
                    a_v = cf[:, :, 0 + 2 * ax_i]
                    nc.vector.tensor_tensor(a_v, w2[:], niv[:], ALU.mult)
                    if ax_i == 0:
                        nc.vector.tensor_tensor(a_v, a_v, lnop[:], ALU.add)
                    b_f = prep.tile([128, NCHUNK], f32, tag="b_f")
                    nc.vector.tensor_tensor(b_f[:], ucl[:], niv[:], ALU.mult)
                    nc.vector.tensor_scalar_mul(cf[:, :, 1 + 2 * ax_i],
                                                b_f[:], -2.0)

                nc.vector.tensor_copy(packed[:, :, 0:4], cf[:])
                sr1 = prep.tile([128, NCHUNK, 4], f32, tag="sr1")
                nc.vector.tensor_tensor(sr1[:], cf[:], packed[:, :, 0:4],
                                        ALU.subtract)
                nc.vector.tensor_copy(packed[:, :, 4:8], sr1[:])
                sr2 = prep.tile([128, NCHUNK, 4], f32, tag="sr2")
                nc.vector.tensor_tensor(sr2[:], sr1[:], packed[:, :, 4:8],
                                        ALU.subtract)
                nc.vector.tensor_copy(packed[:, :, 8:12], sr2[:])
                return packed

            # prep for BOTH poses up front so pose 1's coefficients are ready
            # before the PE reaches them (no mid-kernel stall)
            packed_all = [build_packed(p) for p in range(NPOSE)]

            # transpose ALL blocks of both poses up front: 16 PE transposes
            # into one 2-bank PSUM staging tile per pose, then a single big
            # DVE copy to SBUF.  Removes every per-block PSUM->SBUF copy from
            # the steady-state pipeline (no cross-engine in-order stalls).
            t32all = cpool.tile([128, NPOSE * NBLK, 128], bf16)
            for p in range(NPOSE):
                trs = ps_tr.tile([128, NBLK, 128], bf16, tag="trs",
                                 name=f"trs{p}")
                for bb in range(NBLK):
                    nc.tensor.transpose(
                        trs[:, bb, :], packed_all[p][:, 4 * bb: 4 * bb + 4, :]
                        .rearrange("p a b -> p (a b)"), ident[:])
                nc.vector.tensor_copy(
                    t32all[:, p * NBLK:(p + 1) * NBLK, :], trs[:])

            def args_exp_colors(t32, bb):
                """Arg MMs + fused exp + colors for block bb.  Returns the
                block tile: cols 0:384 = color-scaled Ex, 384:512 = Ex (den),
                512:640 = Ey."""
                parg = ps_arg.tile([128, 1024], f32, tag="arg")
                nc.tensor.matmul(parg[:, 0:512], t32[:], constx[:],
                                 start=True, stop=True)
                nc.tensor.matmul(parg[:, 512:1024], t32[:], consty[:],
                                 start=True, stop=True)
                blk = blkpool.tile([128, 4, 640], f32r, tag="blk")
                # one call: Ex into [:, :, 384:512], Ey into [:, :, 512:640]
                nc.scalar.activation(
                    blk[:, :, 384:640].rearrange("p a (s x) -> p a s x", s=2),
                    parg[:].rearrange("p (s a x) -> p a s x", s=2, a=4),
                    ACTF.Exp)
                # color scale: channels 0,1 on DVE; channel 2 on GPSIMD
                nc.vector.tensor_tensor(
                    blk[:, :, 0:256].rearrange("p a (c x) -> p a c x", c=2),
                    blk[:, :, 384:512].unsqueeze(2)
                    .broadcast_to([128, 4, 2, 128]),
                    col3[:, 4 * bb: 4 * bb + 4, 0:2].unsqueeze(3)
                    .broadcast_to([128, 4, 2, 128]),
                    ALU.mult)
                nc.gpsimd.tensor_tensor(
                    blk[:, :, 256:384].rearrange("p a (c x) -> p a c x", c=1),
                    blk[:, :, 384:512].unsqueeze(2)
                    .broadcast_to([128, 4, 1, 128]),
                    col3[:, 4 * bb: 4 * bb + 4, 2:3].unsqueeze(3)
                    .broadcast_to([128, 4, 1, 128]),
                    ALU.mult)
                return blk

            # flatten the (pose, block) space into one software pipeline:
            # iteration i transposes block i+1, computes args/exp/colors of
            # block i, and runs the main matmuls of block i-1
            units = [(p, bb) for p in range(NPOSE) for bb in range(NBLK)]
            paccs = [ps_acc.tile([128, 512], f32, tag="acc", name=f"pacc{p}")
                     for p in range(NPOSE)]
            def pose_rs(p):
                """Per-pose ReduceScatter trigger, emitted right after pose
                p's accumulator drain.  The gpsimd trigger is fire-and-forget
                (SEQ frees before the collective runs), so pose 0's 15.8us
                collective hides behind pose 1's compute.  NOTHING that waits
                on collective completion may be emitted mid-pipeline: any
                in-order queue (even SP's) head-of-line blocks on it and
                stalls the whole machine."""
                if use_collective:
                    nc.gpsimd.collective_compute(
                        "ReduceScatter", ALU.add,
                        replica_groups=[list(range(NCORES))],
                        ins=[bnc_in[p].opt()],
                        outs=[bnc_out[p].opt()])
                else:
                    nc.sync.dma_start(bnc_out[p], bnc_in[p, 0:RSROWS, :])

            def pose_div(p):
                # tile_wait_until pushes these past all compute in the tile
                # scheduler's clock: they depend on collective completion, and
                # scheduled any earlier they head-of-line block an engine
                # queue behind the 15.8us ReduceScatter
                with tc.tile_wait_until(0.2 + 0.01 * p):
                    sum_sb = fin.tile([RSROWS, 512], f32, tag=f"sum{p}")
                    nc.sync.dma_start(sum_sb[:], bnc_out[p])
                    dplus = fin.tile([RSROWS, 128], f32, tag=f"dplus{p}")
                    nc.vector.tensor_scalar_add(dplus[:], sum_sb[:, 384:512],
                                                float(eps_total))
                    rcp = fin.tile([RSROWS, 128], f32, tag=f"rcp{p}")
                    nc.vector.reciprocal(rcp[:], dplus[:])
                    img = fin.tile([RSROWS, 3, 128], f32, tag=f"img{p}")
                    nc.vector.tensor_tensor(
                        img[:], sum_sb[:, 0:384]
                        .rearrange("p (c x) -> p c x", c=3),
                        rcp[:].unsqueeze(1).broadcast_to([RSROWS, 3, 128]),
                        ALU.mult)
                    nc.sync.dma_start(out.ap()[p].transpose([1, 0, 2]),
                                      img[:])

            def flush_pending(pending):
                """Main MMs of the previous block; on a pose's last block,
                drain its PSUM accumulator to DRAM and emit the pose's whole
                reduce/divide/output tail immediately."""
                pp, pblk, first, last = pending
                for k in range(4):
                    nc.tensor.matmul(
                        paccs[pp][:], pblk[:, k, 512:640],
                        pblk[:, k, 0:512],
                        start=(first and k == 0), stop=(last and k == 3))
                if last:
                    acc_sb = fin.tile([128, 512], f32, tag=f"accsb{pp}")
                    nc.scalar.copy(acc_sb[:], paccs[pp][:])
                    nc.sync.dma_start(bnc_in[pp], acc_sb[:])
                    pose_rs(pp)

            # main MMs run TWO iterations behind args/exp/colors: by then the
            # block's colors are always complete, so PE never stalls on the
            # exp -> colors dependency chain
            from collections import deque
            pend_q = deque()        # (pose, blk tile, is_first, is_last)
            for i, (p, bb) in enumerate(units):
                blk = args_exp_colors(t32all[:, i, :], bb)
                pend_q.append((p, blk, bb == 0, bb == NBLK - 1))
                if len(pend_q) > 2:
                    flush_pending(pend_q.popleft())
            while pend_q:
                flush_pending(pend_q.popleft())
            for p in range(NPOSE):
                pose_div(p)

    nc.compile()
    return nc


_CACHE = {}


def _get_nc(eps_total: float):
    key = float(eps_total)
    if key not in _CACHE:
        _CACHE[key] = _build(key)
    return _CACHE[key]


def kernel(positions, colors, opacities, scales, qvec, tvec,
           tile_hw=32, chunk_gauss=4096):
    positions = np.asarray(positions, np.float32)
    colors = np.asarray(colors, np.float32)
    opacities = np.asarray(opacities, np.float32)
    scales = np.asarray(scales, np.float32)
    qvec = np.asarray(qvec, np.float32)
    tvec = np.asarray(tvec, np.float32)
    tile_hw = int(tile_hw)
    chunk_gauss = int(chunk_gauss)
    n = positions.shape[0]
    assert n == NG and tile_hw == 32, (n, tile_hw)
    eps_total = (n // chunk_gauss) * 1e-8

    rtv = np.zeros((NPOSE, 12), np.float32)
    for p in range(NPOSE):
        rtv[p, :9] = _quat2mat(qvec[p]).astype(np.float32).reshape(9)
        rtv[p, 9:12] = tvec[p]

    def lay(a, shape):
        return np.ascontiguousarray(
            a.reshape(NCHUNK, 128, -1).transpose(1, 0, 2).reshape(shape))

    in_maps = []
    for c in range(NCORES):
        sl = slice(c * NGC, (c + 1) * NGC)
        in_maps.append({
            "positions": lay(positions[sl], (128, NCHUNK, 3)),
            "colors": lay(colors[sl], (128, NCHUNK, 3)),
            "opacities": lay(opacities[sl], (128, NCHUNK)),
            "scales": lay(scales[sl], (128, NCHUNK)),
            "rt": rtv,
        })

    nc = _get_nc(eps_total)
    res = None
    for attempt in range(3):
        try:
            res = run_bass_kernel_spmd(nc, in_maps, core_ids=list(range(NCORES)))
            break
        except Exception:
            if attempt == 2:
                raise
    if res.exec_time_ns is not None:
        print(f"HW exec time: {res.exec_time_ns} ns")
    # core c holds rows 16c..16c+16 of each pose (per-pose RS segment c)
    dev = np.zeros((NPOSE, 3, H, W), np.float32)
    for c in range(NCORES):
        r0 = RSROWS * c
        dev[:, :, r0:r0 + RSROWS, :] = res.results[c]["out"]
    return (dev.reshape(NPOSE, 3, 16, 1024).transpose(0, 2, 1, 3)
            .reshape(NPOSE * 16, 3, tile_hw, tile_hw).astype(np.float32))
